# revision 1
# baseline (speedup 1.0000x reference)
"""GCNAlign 2-layer GCN forward on 8 trn2 NeuronCores.

Strategy (per branch, branches processed on all 8 cores):
  - Nodes are re-numbered (host-side) so that each of the 8*49 node tiles
    (128 nodes each, last tile per core 106) has a near-equal number of
    incoming edges ("balanced tiles").  Core c owns table rows
    [c*6250, (c+1)*6250).
  - x0 = L2-normalized embedding rows, computed on the owning core, then
    AllGather'ed into a full 50000x200 table in every core's HBM.
  - Per target tile: indirect-DMA gather of the (padded) incoming-edge
    source rows [128, CF, 200], then for each chunk of 128 edges build a
    scaled one-hot matrix  oh[p, j] = (tgt_loc[p]==j) * ew[p]  and
    accumulate  u += oh.T @ G_chunk  on the TensorEngine (PSUM).
    u is the aggregated message block [128 nodes, 200].
  - v = u @ W + b via PE (transpose u with the PE, then two accumulating
    matmuls against W rows), relu -> next-layer x tile.
  - AllGather x1 shards, run layer 2 the same way, write output shard.
Host assembles/unpermutes the 8 output shards.
"""

import os
import heapq
import numpy as np

import concourse.bass as bass
import concourse.bacc as bacc
import concourse.mybir as mybir
from concourse.bass import IndirectOffsetOnAxis
from concourse.tile import TileContext
from concourse import bass_utils

F32 = mybir.dt.float32
F16 = mybir.dt.float16
I32 = mybir.dt.int32

N_NODES = 50000
N_EDGES = 800000
DIM = 200
N_CORES = 8
NPC = N_NODES // N_CORES  # 6250 nodes per core


def tile_sizes_for(npc):
    """Per-core tile sizes: full 128-tiles plus one remainder tile."""
    sizes = [128] * (npc // 128)
    if npc % 128:
        sizes.append(npc % 128)
    return sizes


# ---------------------------------------------------------------------------
# Host-side planning: balanced node->tile assignment + per-tile edge layout
# ---------------------------------------------------------------------------

def plan_branch(edges, ew, n_nodes, n_cores, sizes, cf_force=None):
    """Returns dict with perm, idx/tgt/ew arrays [cores, T, 128, CF], cf."""
    src = np.asarray(edges[0], dtype=np.int64)
    tgt = np.asarray(edges[1], dtype=np.int64)
    ew = np.asarray(ew, dtype=np.float32).reshape(-1)
    T = len(sizes)
    n_tiles = n_cores * T
    caps = np.tile(np.asarray(sizes, dtype=np.int64), n_cores)  # [n_tiles]
    assert caps.sum() == n_nodes

    deg = np.bincount(tgt, minlength=n_nodes)
    # Greedy balanced partition: highest-degree nodes first, into the tile
    # with the smallest degree-sum that still has capacity.
    order = np.argsort(-deg, kind="stable")
    heap = [(0, t) for t in range(n_tiles)]
    heapq.heapify(heap)
    remaining = caps.copy()
    tile_of_node = np.empty(n_nodes, dtype=np.int32)
    tile_members = [[] for _ in range(n_tiles)]
    for node in order:
        while True:
            s, t = heapq.heappop(heap)
            if remaining[t] > 0:
                break
        tile_of_node[node] = t
        tile_members[t].append(node)
        remaining[t] -= 1
        if remaining[t] > 0:
            heapq.heappush(heap, (s + int(deg[node]), t))

    # perm: node order = concat of tile member lists (core-major); the table
    # row of original node perm[i] is i.
    perm = np.concatenate([np.asarray(m, dtype=np.int64) for m in tile_members])
    inv_perm = np.empty(n_nodes, dtype=np.int64)
    inv_perm[perm] = np.arange(n_nodes)

    # local row of each node within its tile
    tile_starts_nodes = np.concatenate([[0], np.cumsum(caps)])
    loc_of_node = inv_perm - tile_starts_nodes[tile_of_node]

    # group edges by target tile
    e_tile = tile_of_node[tgt]
    e_order = np.argsort(e_tile, kind="stable")
    e_tile_sorted = e_tile[e_order]
    bounds = np.searchsorted(e_tile_sorted, np.arange(n_tiles + 1))
    counts = bounds[1:] - bounds[:-1]
    cf = int(np.max((counts + 127) // 128)) if cf_force is None else cf_force
    assert int(np.max((counts + 127) // 128)) <= cf, "cf_force too small"
    cf = max(cf, 1)

    pad_e = cf * 128
    idx_a = np.zeros((n_tiles, pad_e), dtype=np.int32)
    tgt_a = np.zeros((n_tiles, pad_e), dtype=np.float32)
    ew_a = np.zeros((n_tiles, pad_e), dtype=np.float32)
    src_new = inv_perm[src]
    loc_tgt = loc_of_node[tgt]
    for t in range(n_tiles):
        sl = e_order[bounds[t]:bounds[t + 1]]
        n = len(sl)
        idx_a[t, :n] = src_new[sl]
        tgt_a[t, :n] = loc_tgt[sl].astype(np.float32)
        ew_a[t, :n] = ew[sl]
        # padding: idx 0 (valid row), tgt_loc 0, ew 0 -> contributes nothing

    # slot (p, c) takes edge j = c*128 + p  ->  reshape [cf,128] + transpose
    def to_pc(a):
        # [cores, T, 128, cf]
        return a.reshape(n_cores, T, cf, 128).transpose(0, 1, 3, 2)

    idx_pc = to_pc(idx_a)   # [cores, T, 128, cf]
    tgt_pc = to_pc(tgt_a)
    ew_pc = to_pc(ew_a)
    # SBUF-resident layouts: idx [cores, 128, T*cf];
    # meta [cores, 128, T*4cf]: per tile t the 4 cf-wide groups are
    # [tgt, ew, -tgt, -ew] (negatives feed the ACT-engine one-hot path).
    idx_flat = np.ascontiguousarray(
        idx_pc.transpose(0, 2, 1, 3).reshape(n_cores, 128, T * cf))
    meta = np.stack([tgt_pc, ew_pc, -tgt_pc, -ew_pc], axis=3)
    meta_flat = np.ascontiguousarray(
        meta.transpose(0, 2, 1, 3, 4).reshape(n_cores, 128, T * 4 * cf))

    return {
        "perm": perm,
        "cf": cf,
        "idx": idx_flat,
        "meta": meta_flat,
        "_tile_lists": (e_order, bounds, src_new, loc_tgt, ew),
    }


def plan_dg(plan, n_nodes, n_cores, T):
    """dma_gather layout: per (tile, half-table) padded chunk groups.

    Returns idx16 [cores,128,C16], meta [cores,128,4*cft_sum], cfa/cfb lists.
    """
    e_order, bounds, src_new, loc_tgt, ew = plan["_tile_lists"]
    half = n_nodes // 2
    n_tiles = n_cores * T
    per = []  # (t%T==tile) lists per (core,tile,half): (idx_local, tgt, ew)
    cf_th = np.zeros((T, 2), np.int64)
    data = {}
    for g in range(n_tiles):
        c, t = g // T, g % T
        sl = e_order[bounds[g]:bounds[g + 1]]
        s = src_new[sl]
        for h in range(2):
            m = (s >= half) if h else (s < half)
            data[(c, t, h)] = (s[m] - h * half, loc_tgt[sl][m].astype(np.float32),
                              ew[sl][m])
            cf_th[t, h] = max(cf_th[t, h], len(data[(c, t, h)][0]))
    # cf_th now holds max edge count per (tile,half); NI = round_up(.,16)
    ni_th = np.maximum(((cf_th + 15) // 16) * 16, 16)
    cf_th = np.maximum((ni_th + 127) // 128, 1)
    C16 = int(ni_th.sum() // 16)
    cft_sum = int(cf_th.sum())
    idx16 = np.zeros((n_cores, 128, C16), np.int16)
    meta = np.zeros((n_cores, 128, 4 * cft_sum), np.float32)
    for c in range(n_cores):
        col = 0
        mcol = 0
        for t in range(T):
            tgts, ews = [], []
            for h in range(2):
                cf = int(cf_th[t, h])
                ni = int(ni_th[t, h])
                n = cf * 128
                il, tl, el = data[(c, t, h)]
                idx = np.zeros(n, np.int16)
                idx[:len(il)] = il
                tg = np.zeros(n, np.float32)
                tg[:len(tl)] = tl
                ee = np.zeros(n, np.float32)
                ee[:len(el)] = el
                slot = idx.reshape(cf, 128).T  # [128, cf]
                kcols = ni // 16
                grid = np.zeros((16, kcols), np.int16)
                for k in range(kcols):
                    grid[:, k] = slot[16 * (k % 8):16 * (k % 8) + 16, k // 8]
                idx16[c, :, col:col + kcols] = np.tile(grid, (8, 1))
                col += kcols
                tgts.append(tg.reshape(cf, 128).T)
                ews.append(ee.reshape(cf, 128).T)
            tg = np.concatenate(tgts, axis=1)  # [128, cft]
            ee = np.concatenate(ews, axis=1)
            cft = tg.shape[1]
            meta[c, :, mcol:mcol + cft] = tg
            meta[c, :, mcol + cft:mcol + 2 * cft] = ee
            meta[c, :, mcol + 2 * cft:mcol + 3 * cft] = -tg
            meta[c, :, mcol + 3 * cft:mcol + 4 * cft] = -ee
            mcol += 4 * cft
    return {"idx16": idx16, "meta": meta,
            "cfa": cf_th[:, 0].tolist(), "cfb": cf_th[:, 1].tolist(),
            "nia": ni_th[:, 0].tolist(), "nib": ni_th[:, 1].tolist()}


# ---------------------------------------------------------------------------
# Bass kernel builder
# ---------------------------------------------------------------------------

def build_gcn(n_cores, n_nodes, sizes, cf, dim=DIM, fp16=True, dg=None):
    npc = n_nodes // n_cores
    T = len(sizes)
    TD = F16 if fp16 else F32
    dpad = 256 if dg else dim
    I16 = mybir.dt.int16
    nc = bacc.Bacc("TRN2", target_bir_lowering=False, debug=False,
                   num_devices=n_cores)
    AT = mybir.ActivationFunctionType
    OP = mybir.AluOpType
    rg = [list(range(n_cores))]

    emb_in, idx_in, meta_in, out_ext = {}, {}, {}, {}
    for br in range(2):
        emb_in[br] = nc.dram_tensor(f"emb{br}", [npc, dim], F32, kind="ExternalInput")
        if dg:
            c16 = (sum(dg[br]["nia"]) + sum(dg[br]["nib"])) // 16
            mw = 4 * (sum(dg[br]["cfa"]) + sum(dg[br]["cfb"]))
            idx_in[br] = nc.dram_tensor(f"idx{br}", [128, c16], I16,
                                        kind="ExternalInput")
            meta_in[br] = nc.dram_tensor(f"meta{br}", [128, mw], F32,
                                         kind="ExternalInput")
        else:
            idx_in[br] = nc.dram_tensor(f"idx{br}", [128, T * cf], I32,
                                        kind="ExternalInput")
            meta_in[br] = nc.dram_tensor(f"meta{br}", [128, T * 4 * cf], F32,
                                         kind="ExternalInput")
        out_ext[br] = nc.dram_tensor(f"out{br}", [npc, dim], F32, kind="ExternalOutput")
    w_in = nc.dram_tensor("conv_w", [dim, dim], F32, kind="ExternalInput")
    b_in = nc.dram_tensor("conv_b", [128, dim], F32, kind="ExternalInput")

    row_slices = []
    off = 0
    for sz in sizes:
        row_slices.append((off, sz))
        off += sz
    assert off == npc

    with TileContext(nc) as tc:
        with (
            tc.tile_pool(name="const", bufs=1) as cpool,
            tc.tile_pool(name="dram", bufs=1, space="DRAM") as dpool,
            tc.tile_pool(name="work", bufs=3) as work,
            tc.tile_pool(name="gbuf", bufs=4) as gpool,
            tc.tile_pool(name="oh", bufs=6) as ohpool,
            tc.tile_pool(name="psum", bufs=2, space="PSUM") as pspool,
            tc.tile_pool(name="outs", bufs=3) as outp,
        ):
            # ---- constants ----
            w_a = cpool.tile([128, dim], F32)
            nc.sync.dma_start(w_a[:], w_in[0:128, :])
            w_b = cpool.tile([dim - 128, dim], F32)
            nc.sync.dma_start(w_b[:], w_in[128:dim, :])
            bb = cpool.tile([128, dim], F32)
            nc.sync.dma_start(bb[:], b_in[:, :])
            iota_i = cpool.tile([128, 128], I32)
            nc.gpsimd.iota(iota_i[:], pattern=[[1, 128]], channel_multiplier=0)
            iota_f = cpool.tile([128, 128], F32)
            nc.vector.tensor_copy(iota_f[:], iota_i[:])
            pidx_i = cpool.tile([128, 1], I32)
            nc.gpsimd.iota(pidx_i[:], pattern=[[0, 1]], channel_multiplier=1)
            pidx_f = cpool.tile([128, 1], F32)
            nc.vector.tensor_copy(pidx_f[:], pidx_i[:])
            ident = cpool.tile([128, 128], F32)
            nc.vector.tensor_scalar(
                out=ident[:], in0=iota_f[:], scalar1=pidx_f[:, :1], scalar2=None,
                op0=OP.is_equal)

            iota_h = cpool.tile([128, 128], F16)
            nc.vector.tensor_copy(iota_h[:], iota_i[:])

            if dg:
                mca = max(max(dg[b]["cfa"]) for b in range(2))
                mcb = max(max(dg[b]["cfb"]) for b in range(2))
                for _i in range(4):
                    za = gpool.tile([128, mca, dpad], TD, tag="GA")
                    nc.vector.memset(za[:], 0.0)
                    zb = gpool.tile([128, mcb, dpad], TD, tag="GB")
                    nc.vector.memset(zb[:], 0.0)

            # resident per-branch edge metadata
            idx_sb, meta_sb = {}, {}
            for br in range(2):
                idx_sb[br] = cpool.tile(list(idx_in[br].shape),
                                        idx_in[br].dtype, name=f"idxsb{br}")
                nc.sync.dma_start(idx_sb[br][:], idx_in[br][:, :])
                meta_sb[br] = cpool.tile(list(meta_in[br].shape), F32,
                                         name=f"metasb{br}")
                nc.sync.dma_start(meta_sb[br][:], meta_in[br][:, :])

            # ---- DRAM bounce/table tiles ----
            x0_shard, x0_tab, x1_shard, x1_tab = {}, {}, {}, {}
            for br in range(2):
                x0_shard[br] = dpool.tile([npc, dpad], TD, name=f"x0s{br}")
                x0_tab[br] = dpool.tile([n_nodes, dpad], TD, addr_space="Shared",
                                        name=f"x0t{br}")
                x1_shard[br] = dpool.tile([npc, dpad], TD, name=f"x1s{br}")
                x1_tab[br] = dpool.tile([n_nodes, dpad], TD, addr_space="Shared",
                                        name=f"x1t{br}")

            def normalize(br):
                for t in range(T):
                    off, sz = row_slices[t]
                    e_t = work.tile([128, dim], F32, tag="nrm_in")
                    nc.sync.dma_start(e_t[:sz], emb_in[br][off:off + sz, :])
                    sq = work.tile([128, dim], F32, tag="nrm_sq")
                    ssq = work.tile([128, 1], F32, tag="nrm_ssq")
                    nc.scalar.activation(sq[:sz], e_t[:sz], AT.Square,
                                         accum_out=ssq[:sz])
                    nrm = work.tile([128, 1], F32, tag="nrm_n")
                    nc.scalar.activation(nrm[:sz], ssq[:sz], AT.Sqrt)
                    nc.vector.tensor_scalar_max(nrm[:sz], nrm[:sz], 1e-12)
                    inv = work.tile([128, 1], F32, tag="nrm_i")
                    nc.vector.reciprocal(inv[:sz], nrm[:sz])
                    xo = outp.tile([128, dpad], TD, tag="nrm_out")
                    if dg:
                        nc.vector.memset(xo[:, dim:dpad], 0.0)
                    nc.vector.tensor_scalar_mul(xo[:sz, 0:dim], e_t[:sz],
                                                inv[:sz, :1])
                    nc.sync.dma_start(x0_shard[br][off:off + sz, :],
                                      xo[:sz, :])

            def allgather(shard, tab, br, name):
                nc.gpsimd.collective_compute(
                    "AllGather", mybir.AluOpType.bypass, replica_groups=rg,
                    ins=[shard[:]], outs=[tab[:]])

            def layer(br, tab, dst, out_dt, wide=False, T=T):
                half = n_nodes // 2
                colo = 0
                mcol = 0
                for t in range(T):
                    off, sz = row_slices[t]
                    if dg:
                        cfa, cfb = dg[br]["cfa"][t], dg[br]["cfb"][t]
                        nia, nib = dg[br]["nia"][t], dg[br]["nib"][t]
                        cft = cfa + cfb
                        ga = gpool.tile([128, cfa, dpad], TD, tag="GA")
                        nc.gpsimd.dma_gather(
                            ga[:], tab[0:half, :],
                            idx_sb[br][:, colo:colo + nia // 16],
                            nia, nia, dpad, single_packet=False)
                        gb = gpool.tile([128, cfb, dpad], TD, tag="GB")
                        nc.gpsimd.dma_gather(
                            gb[:], tab[half:n_nodes, :],
                            idx_sb[br][:, colo + nia // 16:
                                       colo + (nia + nib) // 16],
                            nib, nib, dpad, single_packet=False)
                        colo += (nia + nib) // 16
                        tb = mcol
                        mcol += 4 * cft
                    else:
                        cft = cf
                        g = gpool.tile([128, cf, dim], TD, tag="G")
                        for c in range(cf):
                            nc.gpsimd.indirect_dma_start(
                                out=g[:, c, :], out_offset=None, in_=tab[:],
                                in_offset=IndirectOffsetOnAxis(
                                    ap=idx_sb[br][:, t * cf + c:t * cf + c + 1],
                                    axis=0))
                        tb = 4 * t * cf  # column groups: tgt, ew, -tgt, -ew
                    u = pspool.tile([128, dim], F32, tag="u")
                    ms = meta_sb[br]
                    for c in range(cft):
                        if dg:
                            rhs = (ga[:, c, 0:dim] if c < cfa
                                   else gb[:, c - cfa, 0:dim])
                        else:
                            rhs = g[:, c, :]
                        oh = ohpool.tile([128, 128], TD, tag="oh")
                        if fp16 and c % 2 == 0:
                            # ACT path: |iota - tgt| then relu(ew - ew*|.|)
                            ab = ohpool.tile([128, 128], F16, tag="ab")
                            nc.scalar.activation(
                                ab[:], iota_h[:], AT.Abs,
                                bias=ms[:, tb + 2 * cft + c:tb + 2 * cft + c + 1])
                            nc.scalar.activation(
                                oh[:], ab[:], AT.Relu,
                                bias=ms[:, tb + cft + c:tb + cft + c + 1],
                                scale=ms[:, tb + 3 * cft + c:tb + 3 * cft + c + 1])
                        elif fp16:
                            oh32 = ohpool.tile([128, 128], F32, tag="oh32")
                            nc.vector.tensor_scalar(
                                out=oh32[:], in0=iota_f[:],
                                scalar1=ms[:, tb + c:tb + c + 1],
                                scalar2=ms[:, tb + cft + c:tb + cft + c + 1],
                                op0=mybir.AluOpType.is_equal,
                                op1=mybir.AluOpType.mult)
                            nc.vector.tensor_copy(oh[:], oh32[:])
                        else:
                            nc.vector.tensor_scalar(
                                out=oh[:], in0=iota_f[:],
                                scalar1=ms[:, tb + c:tb + c + 1],
                                scalar2=ms[:, tb + cft + c:tb + cft + c + 1],
                                op0=mybir.AluOpType.is_equal,
                                op1=mybir.AluOpType.mult)
                        nc.tensor.matmul(u[:], lhsT=oh[:], rhs=rhs,
                                         start=(c == 0), stop=(c == cft - 1))
                    u_s = work.tile([128, dim], F32, tag="u_s")
                    nc.vector.tensor_copy(u_s[:], u[:])
                    ut = pspool.tile([128, 256], F32, tag="uT")
                    nc.tensor.transpose(ut[:, 0:128], u_s[:, 0:128], ident[:])
                    nc.tensor.transpose(ut[0:dim - 128, 128:256],
                                        u_s[:, 128:dim], ident[:])
                    ut_s = work.tile([128, 256], F32, tag="uT_s")
                    nc.vector.tensor_copy(ut_s[:, 0:128], ut[:, 0:128])
                    nc.vector.tensor_copy(ut_s[0:dim - 128, 128:256],
                                          ut[0:dim - 128, 128:256])
                    v = pspool.tile([128, dim], F32, tag="v")
                    nc.tensor.matmul(v[:], lhsT=ut_s[:, 0:128], rhs=w_a[:],
                                     start=True, stop=False)
                    nc.tensor.matmul(v[:], lhsT=ut_s[0:dim - 128, 128:256],
                                     rhs=w_b[:], start=False, stop=True)
                    xadd = outp.tile([128, dim], F32, tag="xadd")
                    nc.vector.tensor_tensor(xadd[:], v[:], bb[:],
                                            op=mybir.AluOpType.add)
                    if wide:
                        xo = outp.tile([128, dpad], out_dt, tag="xo")
                        nc.vector.memset(xo[:, dim:dpad], 0.0)
                        nc.scalar.activation(xo[:, 0:dim], xadd[:], AT.Relu)
                        nc.sync.dma_start(dst[off:off + sz, :], xo[:sz, :])
                    else:
                        xo = outp.tile([128, dim], out_dt, tag="xo")
                        nc.scalar.activation(xo[:], xadd[:], AT.Relu)
                        nc.sync.dma_start(dst[off:off + sz, :], xo[:sz])

            for br in range(2):
                normalize(br)
                allgather(x0_shard[br], x0_tab[br], br, "x0")
            for br in range(2):
                layer(br, x0_tab[br], x1_shard[br], TD, wide=bool(dg))
                allgather(x1_shard[br], x1_tab[br], br, "x1")
            for br in range(2):
                layer(br, x1_tab[br], out_ext[br], F32)

    nc.compile()
    return nc


# ---------------------------------------------------------------------------
# Entry point
# ---------------------------------------------------------------------------

def _run(match_emb, ref_emb, conv_w, conv_b, match_edges, ref_edges,
         match_ew, ref_ew, n_nodes, n_cores, trace=False, fp16=True,
         use_dg=False):
    npc = n_nodes // n_cores
    sizes = tile_sizes_for(npc)
    plans = []
    for edges, ew in ((match_edges, match_ew), (ref_edges, ref_ew)):
        plans.append(plan_branch(edges, ew, n_nodes, n_cores, sizes))
    cf = max(p["cf"] for p in plans)
    for i, (edges, ew) in enumerate(((match_edges, match_ew),
                                     (ref_edges, ref_ew))):
        if plans[i]["cf"] != cf:
            plans[i] = plan_branch(edges, ew, n_nodes, n_cores, sizes,
                                   cf_force=cf)

    dg = None
    if use_dg:
        fp16 = True
        dg = {b: plan_dg(plans[b], n_nodes, n_cores, len(sizes))
              for b in range(2)}

    nc = build_gcn(n_cores, n_nodes, sizes, cf, fp16=fp16, dg=dg)

    embs = [np.asarray(match_emb, np.float32), np.asarray(ref_emb, np.float32)]
    emb_perm = [embs[b][plans[b]["perm"]] for b in range(2)]
    b_bcast = np.ascontiguousarray(
        np.broadcast_to(np.asarray(conv_b, np.float32)[None, :], (128, DIM)))
    w_np = np.ascontiguousarray(np.asarray(conv_w, np.float32))

    in_maps = []
    for c in range(n_cores):
        m = {"conv_w": w_np, "conv_b": b_bcast}
        for br in range(2):
            m[f"emb{br}"] = np.ascontiguousarray(
                emb_perm[br][c * npc:(c + 1) * npc])
            if dg is not None:
                m[f"idx{br}"] = np.ascontiguousarray(dg[br]["idx16"][c])
                m[f"meta{br}"] = np.ascontiguousarray(dg[br]["meta"][c])
            else:
                m[f"idx{br}"] = np.ascontiguousarray(plans[br]["idx"][c])
                m[f"meta{br}"] = np.ascontiguousarray(plans[br]["meta"][c])
        in_maps.append(m)

    res = bass_utils.run_bass_kernel_spmd(
        nc, in_maps, core_ids=list(range(n_cores)), trace=trace)

    outs = []
    for br in range(2):
        full = np.empty((n_nodes, DIM), dtype=np.float32)
        perm = plans[br]["perm"]
        for c in range(n_cores):
            full[perm[c * npc:(c + 1) * npc]] = res.results[c][f"out{br}"]
        outs.append(full)
    return (outs[0], outs[1]), res


def kernel(match_emb, ref_emb, conv_w, conv_b, match_edges, ref_edges,
           match_ew, ref_ew):
    trace = bool(int(os.environ.get("KERNEL_TRACE", "0")))
    fp16 = not bool(int(os.environ.get("GCN_FP32", "0")))
    use_dg = bool(int(os.environ.get("GCN_DG", "1")))
    (out_m, out_r), _ = _run(match_emb, ref_emb, conv_w, conv_b,
                             match_edges, ref_edges, match_ew, ref_ew,
                             N_NODES, N_CORES, trace=trace, fp16=fp16,
                             use_dg=use_dg)
    return out_m, out_r

